# revision 38
# baseline (speedup 1.0000x reference)
"""Causal self-attention (B=2, T=2048, C=1024, NH=16) on 8 Trainium2 NeuronCores.

Sharding: core = (batch b, head-group hg): b = core//4, hg = core%4.
Each core handles batch b and 4 heads [4*hg, 4*hg+4) as two head-PAIRS,
computing a partial projection output (w_proj row-parallel). Host sums the
4 partials per batch and adds the (adjusted) bias.

v2 design (vs baseline): everything bf16 on-chip, S^T row-tiled so both
heads of a pair run CONCURRENTLY in the PE array (K=64 each, tile_position
(0,0)/(64,0)), causal-ragged S/exp/PV (only valid columns computed), exp of
both heads in one ACT instruction, denominator ones-column -> DVE reciprocal
-> K=2 indicator broadcast matmul -> in-place yT normalize. The projection
(qk/v) chains are software-pipelined INTO the ACT-paced attention phases as
PE filler so the HAM clock gate stays at K=8/8 (2.4 GHz).
"""

import os
import numpy as np
from contextlib import ExitStack

import concourse.bass as bass
import concourse.tile as tile
from concourse import bacc, mybir
from concourse.bass_utils import run_bass_kernel_spmd

F32 = mybir.dt.float32
F32R = mybir.dt.float32r
BF16 = mybir.dt.bfloat16
EXP = mybir.ActivationFunctionType.Exp
COPY = mybir.ActivationFunctionType.Copy

B, T, C = 2, 2048, 1024
NH, HD = 16, 64
NCORES = 8
HPC = 4            # heads per core
CS = HPC * HD      # 256 channels per core (per q/k/v)
KT = T // 128      # 16 k-tiles
NJ = T // 512      # 4 q-chunks
SCALE = 1.0 / np.sqrt(HD)

_NC_CACHE = None


def _register_ntff_hook():
    """The agent image's ``antenv`` lacks ``axon_hooks``; inject it and
    register the ctypes NTFF profiling hook so trace=True yields timings."""
    try:
        import sys, types, importlib
        if "antenv.axon_hooks" in sys.modules:
            return True
        tb = importlib.import_module("trn_agent_boot.trn_boot")
        hook = tb._ntff_profile_via_ctypes("/opt/axon/libaxon_pjrt.so")
        if hook is None:
            return False
        mod = types.ModuleType("antenv.axon_hooks")
        state = {"hook": hook}
        mod.set_axon_ntff_profile_hook = lambda h: state.update(hook=h)
        mod.get_axon_ntff_profile_hook = lambda: state["hook"]
        sys.modules["antenv.axon_hooks"] = mod
        import antenv
        antenv.axon_hooks = mod
        return True
    except Exception:
        return False


def _build_nc():
    nc = bacc.Bacc("TRN2", target_bir_lowering=False, debug=False)

    xT = nc.dram_tensor("xT", [C, T], BF16, kind="ExternalInput").ap()
    # host-packed [p, k, c] so the big weight DMA is fully contiguous
    wqkv = nc.dram_tensor("wqkv", [128, 8, 3 * CS], BF16, kind="ExternalInput").ap()
    bqk = nc.dram_tensor("bqk", [128, 4], F32, kind="ExternalInput").ap()
    wproj = nc.dram_tensor("wproj", [CS, C], BF16, kind="ExternalInput").ap()
    maskd = nc.dram_tensor("maskd", [128, 256], BF16, kind="ExternalInput").ap()
    ind2 = nc.dram_tensor("ind2", [1, 256], F32R, kind="ExternalInput").ap()
    out = nc.dram_tensor("out", [T, C], BF16, kind="ExternalOutput").ap()
    dbg = os.environ.get("BASS_DEBUG_DUMP")
    if dbg:
        dbg_d = nc.dram_tensor("dbg_d", [8, 1024], F32, kind="ExternalOutput").ap()
        dbg_dinv = nc.dram_tensor("dbg_dinv", [8, 1024], F32, kind="ExternalOutput").ap()
        dbg_yT = nc.dram_tensor("dbg_yT", [2, 128, T], F32, kind="ExternalOutput").ap()

    with tile.TileContext(nc) as tc:
        with ExitStack() as ctx:
            # ---- persistent sbuf ----
            pers = ctx.enter_context(tc.tile_pool(name="pers", bufs=1))
            xT_sb = [pers.tile([128, T], BF16, tag=f"xT{k}", name=f"xT{k}") for k in range(8)]
            w_big = pers.tile([128, 8, 3 * CS], BF16, tag="w_big")
            w_sb = [w_big[:, k, :] for k in range(8)]
            # qkT m-tiles: m0=q(pair0: h0|h1) m1=q(pair1) m2=k(pair0) m3=k(pair1)
            qkT = [pers.tile([128, T], BF16, tag=f"qkT{m}", name=f"qkT{m}") for m in range(4)]
            # v_aug: [128 k-rows, head, kt, 65]; col 64 = ones (denominator)
            v_sb = pers.tile([128, HPC, KT, 65], BF16, tag="v_sb")
            yT = [pers.tile([128, T], BF16, tag=f"yT{p}", name=f"yT{p}") for p in range(2)]
            wproj_sb = [pers.tile([128, C], BF16, tag=f"wproj{p}", name=f"wproj{p}") for p in range(2)]
            bqk_sb = pers.tile([128, 4], F32, tag="bqk_sb")
            maskd_sb = pers.tile([128, 256], BF16, tag="maskd_sb")
            # row 64 only: keeps the 1/d path lane-aligned with the po
            # denominator row (custom-DVE ops cannot cross partitions)
            ind2_sb = pers.tile([65, 256], F32R, tag="ind2_sb")

            nc.vector.memset(v_sb[:, :, :, 64], 1.0)
            # Two HWDGE rings in parallel (SP + ACT); each dma_start costs
            # ~0.6us of serialized dispatch on its ring, so coalesce.
            nc.sync.dma_start(w_big[:], wqkv[:])
            for k in range(8):   # per-k so the first qk chain trickles early;
                # gpsimd SWDGE ring dispatches immediately (SP busy with w,
                # ACT busy with its preamble + table load)
                nc.gpsimd.dma_start(xT_sb[k][:], xT[k * 128:(k + 1) * 128, :])
            for p in range(2):
                nc.sync.dma_start(wproj_sb[p][:], wproj[p * 128:(p + 1) * 128, :])
            nc.sync.dma_start(bqk_sb[:], bqk[:])
            nc.sync.dma_start(maskd_sb[:], maskd[:])
            nc.sync.dma_start(ind2_sb[64:65, :], ind2[:])

            # ---- pools ----
            att = ctx.enter_context(tc.tile_pool(name="att", bufs=1))
            ctx2 = ctx.enter_context(ExitStack())
            psum = ctx2.enter_context(tc.tile_pool(name="psum", bufs=1, space="PSUM"))
            # psum budget (8 banks): spair 2x2 + po 2x1 + fill 2x1 = 8

            # ---- emission helpers ----
            def emit_qk_chain(m, j):
                pq = psum.tile([128, 512], F32, tag="fill", bufs=2, name=f"pq_{m}_{j}")
                for k in range(8):
                    nc.tensor.matmul(
                        pq[:],
                        w_sb[k][:, m * 128:(m + 1) * 128],
                        xT_sb[k][:, j * 512:(j + 1) * 512],
                        start=(k == 0), stop=(k == 7),
                    )
                nc.vector.tensor_scalar_add(
                    qkT[m][:, j * 512:(j + 1) * 512], pq[:], bqk_sb[:, m:m + 1]
                )

            def emit_v_chain(t):
                pv = psum.tile([128, 256], F32, tag="fill", bufs=2, name=f"pv_{t}")
                for k in range(8):
                    nc.tensor.matmul(
                        pv[:],
                        xT_sb[k][:, t * 128:(t + 1) * 128],
                        w_sb[k][:, 2 * CS:3 * CS],
                        start=(k == 0), stop=(k == 7),
                    )
                nc.vector.tensor_copy(
                    v_sb[:, :, t, 0:64],
                    pv[:].rearrange("p (h d) -> p h d", h=HPC),
                )

            dpool = ctx.enter_context(tc.tile_pool(name="dpool", bufs=1))

            def emit_norm(p, j, po_t):
                """PV for (p,j) done: reciprocal of the two denominator rows
                (fast Newton approx, ~18 bits), and evacuate unnormalized O^T
                into yT (bf16)."""
                dscr = dpool.tile([65, 1024], F32, tag="dscr", bufs=2, name=f"dscr_{p}_{j}")
                dinv = dpool.tile([65, 1024], F32R, tag="dinv", bufs=2, name=f"dinv_{p}_{j}")
                for h2 in range(2):
                    # tracked copy first: strict-FIFO DVE then guarantees the
                    # (custom-op) approx below sees the completed accumulation
                    nc.vector.tensor_copy(
                        yT[p][64 * h2:64 * h2 + 64, j * 512:(j + 1) * 512],
                        po_t[h2][0:64, :],
                    )
                    # full 65-partition approx: base partition 0 (custom-DVE
                    # ops are lane-fixed and only correct at base 0); rows
                    # 0-63 produce unused 1/O junk, row 64 = 1/d.
                    nc.vector.reciprocal_approx_fast(
                        dscr[:, 512 * h2:512 * h2 + 512], po_t[h2][:, :]
                    )
                with nc.allow_low_precision(reason="1/d fits tf32"):
                    nc.vector.tensor_copy(dinv[64:65, :], dscr[64:65, :])
                if dbg:
                    dr = dpool.tile([65, 1024], F32, tag="dbgd", bufs=2, name=f"dr_{p}_{j}")
                    for h2 in range(2):
                        nc.vector.tensor_copy(
                            dr[64:65, 512 * h2:512 * h2 + 512], po_t[h2][64:65, :])
                    nc.sync.dma_start(dbg_d[4 * p + j:4 * p + j + 1, :], dr[64:65, :])
                    nc.sync.dma_start(
                        dbg_dinv[4 * p + j:4 * p + j + 1, :], dscr[64:65, :])
                return dinv

            def emit_norm2(p, j, dinv):
                """Broadcast 1/d across the pair's 128 channel rows (K=2
                indicator matmul) and normalize yT in place."""
                db = psum.tile([128, 512], F32, tag="fill", bufs=2, name=f"db_{p}_{j}")
                for h2 in range(2):
                    nc.tensor.matmul(
                        db[:],
                        ind2_sb[64:65, 128 * h2:128 * h2 + 128],
                        dinv[64:65, 512 * h2:512 * h2 + 512],
                        start=(h2 == 0), stop=(h2 == 1),
                    )
                nc.vector.tensor_mul(
                    yT[p][:, j * 512:(j + 1) * 512],
                    yT[p][:, j * 512:(j + 1) * 512],
                    db[:],
                )

            # ---- attention (per pair), with PE filler interleave ----
            mask3 = maskd_sb[:].rearrange("p (c b) -> p c b", c=2)

            def attention_pair(p, fillers, pending_norm2):
                """fillers: list of (need_step, thunk), sorted by need_step.
                Popped when due (data needed soon) or on a 1-in-3 step pace
                to keep the PE stream dense through the ACT-paced phase."""
                step = 0
                for j in range(NJ):
                    last = 4 * j + 3
                    po_t = None
                    pend = None
                    for kt in range(last + 1):
                        while fillers and fillers[0][0] <= step:
                            fillers.pop(0)[1]()
                        d = max(0, kt - 4 * j)
                        w = 512 - 128 * d
                        qoff = j * 512 + 128 * d
                        spair = psum.tile([128, 1024], F32, tag="spair", bufs=2,
                                          name=f"sp_{p}_{j}_{kt}")
                        sp3 = spair.rearrange("p (c b) -> p c b", c=2)
                        for h2 in range(2):
                            nc.tensor.matmul(
                                sp3[:, h2, 0:w],
                                qkT[2 + p][64 * h2:64 * h2 + 64, kt * 128:(kt + 1) * 128],
                                qkT[p][64 * h2:64 * h2 + 64, qoff:qoff + w],
                                start=True, stop=True,
                                tile_position=(64 * h2, 0),
                            )
                        # flush pending PV (from kt-1) while exp(kt) runs
                        if pend is not None:
                            kt0, pt0, w0, d0 = pend
                            for h2 in range(2):
                                nc.tensor.matmul(
                                    po_t[h2][:, 128 * d0:512],
                                    v_sb[:, 2 * p + h2, kt0, :],
                                    pt0[:, h2, 0:w0],
                                    start=(kt0 == 0), stop=(kt0 == last),
                                )
                            pend = None
                        if step % 3 == 1 and fillers:
                            fillers.pop(0)[1]()
                        if kt == 2 and pending_norm2:
                            pending_norm2.pop(0)()
                        pt = att.tile([128, 1024], BF16, tag="pt", bufs=3,
                                      name=f"pt_{p}_{j}_{kt}")
                        pt3 = pt.rearrange("p (c b) -> p c b", c=2)
                        nc.scalar.activation(pt3[:, :, 0:w], sp3[:, :, 0:w], EXP, scale=SCALE)
                        if kt >= 4 * j:  # diagonal block: triangular mask
                            nc.vector.tensor_mul(
                                pt3[:, :, 0:128], pt3[:, :, 0:128], mask3
                            )
                        if po_t is None:
                            po_t = [psum.tile([65, 512], F32, tag="po", bufs=2,
                                              name=f"po_{p}_{j}_{h2}")
                                    for h2 in range(2)]
                        pend = (kt, pt3, w, d)
                        step += 1
                    # flush last PV of this j
                    kt0, pt0, w0, d0 = pend
                    for h2 in range(2):
                        nc.tensor.matmul(
                            po_t[h2][:, 128 * d0:512],
                            v_sb[:, 2 * p + h2, kt0, :],
                            pt0[:, h2, 0:w0],
                            start=(kt0 == 0), stop=(kt0 == last),
                        )
                    dinv = emit_norm(p, j, po_t)
                    pending_norm2.append(lambda p=p, j=j, dinv=dinv: emit_norm2(p, j, dinv))

            # ---- phase 1 (minimal): only what A0's first steps consume ----
            emit_qk_chain(0, 0)
            emit_qk_chain(2, 0)
            for t in range(4):
                emit_v_chain(t)

            pending_norm2 = []
            # step(j, kt) = base(j) + kt;  base = [0, 4, 12, 24]
            base = [0, 4, 12, 24]
            # ---- A0: pair0 attention; fillers = v[4..15] ----
            # v(t) first consumed by PV(kt=t) at step base(j0)+t+1
            fillers0 = []
            for t in range(4, KT):
                j0 = t // 4  # first j whose kt range reaches t
                fillers0.append((base[j0] + t - 1, lambda t=t: emit_v_chain(t)))
            # pair0's remaining qk chains, just-in-time (qkT[0] chunk c read
            # by S(c,0); qkT[2] chunk c read by S(c,4c))
            for c in range(1, NJ):
                fillers0.append((base[c] - 2, lambda c=c: emit_qk_chain(0, c)))
                fillers0.append((base[c] + 4 * c - 2, lambda c=c: emit_qk_chain(2, c)))
            # pair1's first qk chunks late in A0 so A1 starts without a stall
            fillers0.append((28, lambda: emit_qk_chain(1, 0)))
            fillers0.append((31, lambda: emit_qk_chain(3, 0)))
            fillers0.sort(key=lambda x: x[0])
            attention_pair(0, fillers0, pending_norm2)
            for _, f in fillers0:
                f()
            # ---- A1: pair1 attention; fillers = remaining qk chains ----
            # qk(1,c) read by S(c, 0); qk(3,c) read by S(c, 4c)
            fillers1 = []
            for c in range(1, NJ):
                fillers1.append((base[c] - 2, lambda c=c: emit_qk_chain(1, c)))
                fillers1.append((base[c] + 4 * c - 2, lambda c=c: emit_qk_chain(3, c)))
            fillers1.sort(key=lambda x: x[0])
            attention_pair(1, fillers1, pending_norm2)
            for _, f in fillers1:
                f()

            if dbg:
                for p in range(2):
                    yf = att.tile([128, T], F32, tag="dbgy", bufs=1, name=f"yf_{p}")
                    nc.vector.tensor_copy(yf[:], yT[p][:])
                    nc.sync.dma_start(dbg_yT[p, :, :], yf[:])

            # ---- phase 3: projection (contraction over both pairs) ----
            # pp reuses the attention "spair" psum tag (pools stay open so the
            # deferred norm2(1,3) db matmul can still allocate from "fill")
            def emit_proj(t):
                ob = att.tile([128, C], BF16, tag="ob", bufs=4, name=f"ob_{t}")
                for n in range(2):
                    pp = psum.tile([128, 512], F32, tag="spair", bufs=2, name=f"pp_{t}_{n}")
                    for p in range(2):
                        nc.tensor.matmul(
                            pp[:],
                            yT[p][:, t * 128:(t + 1) * 128],
                            wproj_sb[p][:, n * 512:(n + 1) * 512],
                            start=(p == 0), stop=(p == 1),
                        )
                    if n == 0:
                        nc.vector.tensor_copy(ob[:, n * 512:(n + 1) * 512], pp[:])
                    else:
                        nc.scalar.activation(ob[:, n * 512:(n + 1) * 512], pp[:], COPY)
                # avoid the scalar ring: ACT's queue is backlogged with exps,
                # so its DMA dispatches would all bunch up at the very end
                eng = (nc.sync, nc.gpsimd)[t % 2]
                eng.dma_start(out[t * 128:(t + 1) * 128, :], ob[:])

            for t in range(4):
                emit_proj(t)
            for f in pending_norm2:  # norm2(1,3): after proj t0-3, before t12
                f()
            pending_norm2.clear()
            for t in range(4, KT):
                emit_proj(t)

    nc.compile()
    return nc


def _get_nc():
    global _NC_CACHE
    if _NC_CACHE is None:
        _NC_CACHE = _build_nc()
    return _NC_CACHE


def kernel(x, w_attn, b_attn, w_proj, b_proj, n_heads):
    import ml_dtypes
    bf16 = ml_dtypes.bfloat16

    x = np.asarray(x, dtype=np.float32)
    w_attn = np.asarray(w_attn, dtype=np.float32)
    b_attn = np.asarray(b_attn, dtype=np.float32)
    w_proj = np.asarray(w_proj, dtype=np.float32)
    b_proj = np.asarray(b_proj, dtype=np.float32)
    assert int(n_heads) == NH and x.shape == (B, T, C)

    # triangle: valid iff q - k = f - p >= 0 within the diagonal 128-block
    p_ = np.arange(128)[:, None]
    f_ = np.arange(128)[None, :]
    m1 = (f_ >= p_).astype(np.float32)
    maskd = np.ascontiguousarray(
        np.concatenate([m1, m1], axis=1).astype(bf16))
    ind2 = np.zeros((1, 256), dtype=np.float32)
    ind2[0, 0:64] = 1.0       # cols 0-127: indicator for h0 (rows 0-63)
    ind2[0, 192:256] = 1.0    # cols 128-255: indicator for h1 (rows 64-127)

    in_maps = []
    for core in range(NCORES):
        b, hg = core // 4, core % 4
        cs = hg * CS
        wq = w_attn[:, cs:cs + CS]
        wk = w_attn[:, C + cs:C + cs + CS]
        wv = w_attn[:, 2 * C + cs:2 * C + cs + CS]
        bq = b_attn[cs:cs + CS]
        bk = b_attn[C + cs:C + cs + CS]
        in_maps.append({
            "xT": np.ascontiguousarray(x[b].T.astype(bf16)),
            "wqkv": np.ascontiguousarray(
                np.concatenate([wq, wk, wv], axis=1).astype(bf16)
                .reshape(8, 128, 3 * CS).transpose(1, 0, 2)),
            "bqk": np.ascontiguousarray(
                np.stack([bq[:128], bq[128:], bk[:128], bk[128:]], axis=1)),
            "wproj": np.ascontiguousarray(w_proj[cs:cs + CS, :].astype(bf16)),
            "maskd": maskd,
            "ind2": ind2,
        })

    nc = _get_nc()
    trace = bool(os.environ.get("BASS_TRACE")) and _register_ntff_hook()
    res = run_bass_kernel_spmd(
        nc, in_maps, core_ids=list(range(NCORES)), trace=trace,
    )
    globals()["_LAST_RESULTS"] = res

    # host gather: sum head-group partials per batch, add adjusted bias
    # (v-bias folds through attention+proj into a constant row: b_v @ w_proj)
    b_eff = (b_proj.astype(np.float64)
             + b_attn[2 * C:].astype(np.float64) @ w_proj.astype(np.float64))
    outp = np.zeros((B, T, C), dtype=np.float64)
    for core in range(NCORES):
        outp[core // 4] += np.asarray(res.results[core]["out"]).astype(np.float64)
    outp += b_eff[None, None, :]
    return outp.astype(np.float32)


# revision 39
# speedup vs baseline: 1.0180x; 1.0180x over previous
"""Causal self-attention (B=2, T=2048, C=1024, NH=16) on 8 Trainium2 NeuronCores.

Sharding: core = (batch b, head-group hg): b = core//4, hg = core%4.
Each core handles batch b and 4 heads [4*hg, 4*hg+4) as two head-PAIRS,
computing a partial projection output (w_proj row-parallel). Host sums the
4 partials per batch and adds the (adjusted) bias.

v2 design (vs baseline): everything bf16 on-chip, S^T row-tiled so both
heads of a pair run CONCURRENTLY in the PE array (K=64 each, tile_position
(0,0)/(64,0)), causal-ragged S/exp/PV (only valid columns computed), exp of
both heads in one ACT instruction, denominator ones-column -> DVE reciprocal
-> K=2 indicator broadcast matmul -> in-place yT normalize. The projection
(qk/v) chains are software-pipelined INTO the ACT-paced attention phases as
PE filler so the HAM clock gate stays at K=8/8 (2.4 GHz).
"""

import os
import numpy as np
from contextlib import ExitStack

import concourse.bass as bass
import concourse.tile as tile
from concourse import bacc, mybir
from concourse.bass_utils import run_bass_kernel_spmd

F32 = mybir.dt.float32
F32R = mybir.dt.float32r
BF16 = mybir.dt.bfloat16
EXP = mybir.ActivationFunctionType.Exp
COPY = mybir.ActivationFunctionType.Copy

B, T, C = 2, 2048, 1024
NH, HD = 16, 64
NCORES = 8
HPC = 4            # heads per core
CS = HPC * HD      # 256 channels per core (per q/k/v)
KT = T // 128      # 16 k-tiles
NJ = T // 512      # 4 q-chunks
SCALE = 1.0 / np.sqrt(HD)

_NC_CACHE = None


def _register_ntff_hook():
    """The agent image's ``antenv`` lacks ``axon_hooks``; inject it and
    register the ctypes NTFF profiling hook so trace=True yields timings."""
    try:
        import sys, types, importlib
        if "antenv.axon_hooks" in sys.modules:
            return True
        tb = importlib.import_module("trn_agent_boot.trn_boot")
        hook = tb._ntff_profile_via_ctypes("/opt/axon/libaxon_pjrt.so")
        if hook is None:
            return False
        mod = types.ModuleType("antenv.axon_hooks")
        state = {"hook": hook}
        mod.set_axon_ntff_profile_hook = lambda h: state.update(hook=h)
        mod.get_axon_ntff_profile_hook = lambda: state["hook"]
        sys.modules["antenv.axon_hooks"] = mod
        import antenv
        antenv.axon_hooks = mod
        return True
    except Exception:
        return False


def _build_nc():
    nc = bacc.Bacc("TRN2", target_bir_lowering=False, debug=False)

    xT = nc.dram_tensor("xT", [C, T], BF16, kind="ExternalInput").ap()
    # host-packed [p, k, c] so the big weight DMA is fully contiguous
    wqkv = nc.dram_tensor("wqkv", [128, 8, 3 * CS], BF16, kind="ExternalInput").ap()
    bqk = nc.dram_tensor("bqk", [128, 4], F32, kind="ExternalInput").ap()
    wproj = nc.dram_tensor("wproj", [CS, C], BF16, kind="ExternalInput").ap()
    maskd = nc.dram_tensor("maskd", [128, 256], BF16, kind="ExternalInput").ap()
    ind2 = nc.dram_tensor("ind2", [1, 256], F32R, kind="ExternalInput").ap()
    out = nc.dram_tensor("out", [T, C], BF16, kind="ExternalOutput").ap()
    dbg = os.environ.get("BASS_DEBUG_DUMP")
    if dbg:
        dbg_d = nc.dram_tensor("dbg_d", [8, 1024], F32, kind="ExternalOutput").ap()
        dbg_dinv = nc.dram_tensor("dbg_dinv", [8, 1024], F32, kind="ExternalOutput").ap()
        dbg_yT = nc.dram_tensor("dbg_yT", [2, 128, T], F32, kind="ExternalOutput").ap()

    with tile.TileContext(nc) as tc:
        with ExitStack() as ctx:
            # ---- persistent sbuf ----
            pers = ctx.enter_context(tc.tile_pool(name="pers", bufs=1))
            xT_sb = [pers.tile([128, T], BF16, tag=f"xT{k}", name=f"xT{k}") for k in range(8)]
            w_big = pers.tile([128, 8, 3 * CS], BF16, tag="w_big")
            w_sb = [w_big[:, k, :] for k in range(8)]
            # qkT m-tiles: m0=q(pair0: h0|h1) m1=q(pair1) m2=k(pair0) m3=k(pair1)
            qkT = [pers.tile([128, T], BF16, tag=f"qkT{m}", name=f"qkT{m}") for m in range(4)]
            # v_aug: [128 k-rows, head, kt, 65]; col 64 = ones (denominator)
            v_sb = pers.tile([128, HPC, KT, 65], BF16, tag="v_sb")
            yT = [pers.tile([128, T], BF16, tag=f"yT{p}", name=f"yT{p}") for p in range(2)]
            wproj_sb = [pers.tile([128, C], BF16, tag=f"wproj{p}", name=f"wproj{p}") for p in range(2)]
            bqk_sb = pers.tile([128, 4], F32, tag="bqk_sb")
            maskd_sb = pers.tile([128, 256], BF16, tag="maskd_sb")
            # row 64 only: keeps the 1/d path lane-aligned with the po
            # denominator row (custom-DVE ops cannot cross partitions)
            ind2_sb = pers.tile([65, 256], F32R, tag="ind2_sb")

            nc.vector.memset(v_sb[:, :, :, 64], 1.0)
            # Two HWDGE rings in parallel (SP + ACT); each dma_start costs
            # ~0.6us of serialized dispatch on its ring, so coalesce.
            for k in range(8):   # per-k chunks: the k=0 slices land early so
                # the first qk chain (and attention) starts ASAP
                nc.sync.dma_start(w_big[:, k, :], wqkv[:, k, :])
                (nc.gpsimd, nc.scalar)[k % 2].dma_start(
                    xT_sb[k][:], xT[k * 128:(k + 1) * 128, :])
            for p in range(2):
                nc.sync.dma_start(wproj_sb[p][:], wproj[p * 128:(p + 1) * 128, :])
            nc.sync.dma_start(bqk_sb[:], bqk[:])
            nc.sync.dma_start(maskd_sb[:], maskd[:])
            nc.sync.dma_start(ind2_sb[64:65, :], ind2[:])

            # ---- pools ----
            att = ctx.enter_context(tc.tile_pool(name="att", bufs=1))
            ctx2 = ctx.enter_context(ExitStack())
            psum = ctx2.enter_context(tc.tile_pool(name="psum", bufs=1, space="PSUM"))
            # psum budget (8 banks): spair 2x2 + po 2x1 + fill 2x1 = 8

            # ---- emission helpers ----
            def emit_qk_chain(m, j):
                pq = psum.tile([128, 512], F32, tag="fill", bufs=2, name=f"pq_{m}_{j}")
                for k in range(8):
                    nc.tensor.matmul(
                        pq[:],
                        w_sb[k][:, m * 128:(m + 1) * 128],
                        xT_sb[k][:, j * 512:(j + 1) * 512],
                        start=(k == 0), stop=(k == 7),
                    )
                nc.vector.tensor_scalar_add(
                    qkT[m][:, j * 512:(j + 1) * 512], pq[:], bqk_sb[:, m:m + 1]
                )

            def emit_v_chain(t):
                pv = psum.tile([128, 256], F32, tag="fill", bufs=2, name=f"pv_{t}")
                for k in range(8):
                    nc.tensor.matmul(
                        pv[:],
                        xT_sb[k][:, t * 128:(t + 1) * 128],
                        w_sb[k][:, 2 * CS:3 * CS],
                        start=(k == 0), stop=(k == 7),
                    )
                nc.vector.tensor_copy(
                    v_sb[:, :, t, 0:64],
                    pv[:].rearrange("p (h d) -> p h d", h=HPC),
                )

            dpool = ctx.enter_context(tc.tile_pool(name="dpool", bufs=1))

            def emit_norm(p, j, po_t):
                """PV for (p,j) done: reciprocal of the two denominator rows
                (fast Newton approx, ~18 bits), and evacuate unnormalized O^T
                into yT (bf16)."""
                dscr = dpool.tile([65, 1024], F32, tag="dscr", bufs=2, name=f"dscr_{p}_{j}")
                dinv = dpool.tile([65, 1024], F32R, tag="dinv", bufs=2, name=f"dinv_{p}_{j}")
                for h2 in range(2):
                    # tracked copy first: strict-FIFO DVE then guarantees the
                    # (custom-op) approx below sees the completed accumulation
                    nc.vector.tensor_copy(
                        yT[p][64 * h2:64 * h2 + 64, j * 512:(j + 1) * 512],
                        po_t[h2][0:64, :],
                    )
                    # full 65-partition approx: base partition 0 (custom-DVE
                    # ops are lane-fixed and only correct at base 0); rows
                    # 0-63 produce unused 1/O junk, row 64 = 1/d.
                    nc.vector.reciprocal_approx_fast(
                        dscr[:, 512 * h2:512 * h2 + 512], po_t[h2][:, :]
                    )
                with nc.allow_low_precision(reason="1/d fits tf32"):
                    nc.vector.tensor_copy(dinv[64:65, :], dscr[64:65, :])
                if dbg:
                    dr = dpool.tile([65, 1024], F32, tag="dbgd", bufs=2, name=f"dr_{p}_{j}")
                    for h2 in range(2):
                        nc.vector.tensor_copy(
                            dr[64:65, 512 * h2:512 * h2 + 512], po_t[h2][64:65, :])
                    nc.sync.dma_start(dbg_d[4 * p + j:4 * p + j + 1, :], dr[64:65, :])
                    nc.sync.dma_start(
                        dbg_dinv[4 * p + j:4 * p + j + 1, :], dscr[64:65, :])
                return dinv

            def emit_norm2(p, j, dinv):
                """Broadcast 1/d across the pair's 128 channel rows (K=2
                indicator matmul) and normalize yT in place."""
                db = psum.tile([128, 512], F32, tag="fill", bufs=2, name=f"db_{p}_{j}")
                for h2 in range(2):
                    nc.tensor.matmul(
                        db[:],
                        ind2_sb[64:65, 128 * h2:128 * h2 + 128],
                        dinv[64:65, 512 * h2:512 * h2 + 512],
                        start=(h2 == 0), stop=(h2 == 1),
                    )
                nc.vector.tensor_mul(
                    yT[p][:, j * 512:(j + 1) * 512],
                    yT[p][:, j * 512:(j + 1) * 512],
                    db[:],
                )

            # ---- attention (per pair), with PE filler interleave ----
            mask3 = maskd_sb[:].rearrange("p (c b) -> p c b", c=2)

            def attention_pair(p, fillers, pending_norm2):
                """fillers: list of (need_step, thunk), sorted by need_step.
                Popped when due (data needed soon) or on a 1-in-3 step pace
                to keep the PE stream dense through the ACT-paced phase."""
                step = 0
                for j in range(NJ):
                    last = 4 * j + 3
                    po_t = None
                    pend = None
                    for kt in range(last + 1):
                        while fillers and fillers[0][0] <= step:
                            fillers.pop(0)[1]()
                        d = max(0, kt - 4 * j)
                        w = 512 - 128 * d
                        qoff = j * 512 + 128 * d
                        spair = psum.tile([128, 1024], F32, tag="spair", bufs=2,
                                          name=f"sp_{p}_{j}_{kt}")
                        sp3 = spair.rearrange("p (c b) -> p c b", c=2)
                        for h2 in range(2):
                            nc.tensor.matmul(
                                sp3[:, h2, 0:w],
                                qkT[2 + p][64 * h2:64 * h2 + 64, kt * 128:(kt + 1) * 128],
                                qkT[p][64 * h2:64 * h2 + 64, qoff:qoff + w],
                                start=True, stop=True,
                                tile_position=(64 * h2, 0),
                            )
                        # flush pending PV (from kt-1) while exp(kt) runs
                        if pend is not None:
                            kt0, pt0, w0, d0 = pend
                            for h2 in range(2):
                                nc.tensor.matmul(
                                    po_t[h2][:, 128 * d0:512],
                                    v_sb[:, 2 * p + h2, kt0, :],
                                    pt0[:, h2, 0:w0],
                                    start=(kt0 == 0), stop=(kt0 == last),
                                )
                            pend = None
                        if step % 3 == 1 and fillers:
                            fillers.pop(0)[1]()
                        if kt == 2 and pending_norm2:
                            pending_norm2.pop(0)()
                        pt = att.tile([128, 1024], BF16, tag="pt", bufs=3,
                                      name=f"pt_{p}_{j}_{kt}")
                        pt3 = pt.rearrange("p (c b) -> p c b", c=2)
                        nc.scalar.activation(pt3[:, :, 0:w], sp3[:, :, 0:w], EXP, scale=SCALE)
                        if kt >= 4 * j:  # diagonal block: triangular mask
                            nc.vector.tensor_mul(
                                pt3[:, :, 0:128], pt3[:, :, 0:128], mask3
                            )
                        if po_t is None:
                            po_t = [psum.tile([65, 512], F32, tag="po", bufs=2,
                                              name=f"po_{p}_{j}_{h2}")
                                    for h2 in range(2)]
                        pend = (kt, pt3, w, d)
                        step += 1
                    # flush last PV of this j
                    kt0, pt0, w0, d0 = pend
                    for h2 in range(2):
                        nc.tensor.matmul(
                            po_t[h2][:, 128 * d0:512],
                            v_sb[:, 2 * p + h2, kt0, :],
                            pt0[:, h2, 0:w0],
                            start=(kt0 == 0), stop=(kt0 == last),
                        )
                    dinv = emit_norm(p, j, po_t)
                    pending_norm2.append(lambda p=p, j=j, dinv=dinv: emit_norm2(p, j, dinv))

            # ---- phase 1 (minimal): only what A0's first steps consume ----
            emit_qk_chain(0, 0)
            emit_qk_chain(2, 0)
            for t in range(4):
                emit_v_chain(t)

            pending_norm2 = []
            # step(j, kt) = base(j) + kt;  base = [0, 4, 12, 24]
            base = [0, 4, 12, 24]
            # ---- A0: pair0 attention; fillers = v[4..15] ----
            # v(t) first consumed by PV(kt=t) at step base(j0)+t+1
            fillers0 = []
            for t in range(4, KT):
                j0 = t // 4  # first j whose kt range reaches t
                fillers0.append((base[j0] + t - 1, lambda t=t: emit_v_chain(t)))
            # pair0's remaining qk chains, just-in-time (qkT[0] chunk c read
            # by S(c,0); qkT[2] chunk c read by S(c,4c))
            for c in range(1, NJ):
                fillers0.append((base[c] - 2, lambda c=c: emit_qk_chain(0, c)))
                fillers0.append((base[c] + 4 * c - 2, lambda c=c: emit_qk_chain(2, c)))
            # pair1's first qk chunks late in A0 so A1 starts without a stall
            fillers0.append((28, lambda: emit_qk_chain(1, 0)))
            fillers0.append((31, lambda: emit_qk_chain(3, 0)))
            fillers0.sort(key=lambda x: x[0])
            attention_pair(0, fillers0, pending_norm2)
            for _, f in fillers0:
                f()
            # ---- A1: pair1 attention; fillers = remaining qk chains ----
            # qk(1,c) read by S(c, 0); qk(3,c) read by S(c, 4c)
            fillers1 = []
            for c in range(1, NJ):
                fillers1.append((base[c] - 2, lambda c=c: emit_qk_chain(1, c)))
                fillers1.append((base[c] + 4 * c - 2, lambda c=c: emit_qk_chain(3, c)))
            fillers1.sort(key=lambda x: x[0])
            attention_pair(1, fillers1, pending_norm2)
            for _, f in fillers1:
                f()

            if dbg:
                for p in range(2):
                    yf = att.tile([128, T], F32, tag="dbgy", bufs=1, name=f"yf_{p}")
                    nc.vector.tensor_copy(yf[:], yT[p][:])
                    nc.sync.dma_start(dbg_yT[p, :, :], yf[:])

            # ---- phase 3: projection (contraction over both pairs) ----
            # pp reuses the attention "spair" psum tag (pools stay open so the
            # deferred norm2(1,3) db matmul can still allocate from "fill")
            def emit_proj(t):
                ob = att.tile([128, C], BF16, tag="ob", bufs=4, name=f"ob_{t}")
                for n in range(2):
                    pp = psum.tile([128, 512], F32, tag="spair", bufs=2, name=f"pp_{t}_{n}")
                    for p in range(2):
                        nc.tensor.matmul(
                            pp[:],
                            yT[p][:, t * 128:(t + 1) * 128],
                            wproj_sb[p][:, n * 512:(n + 1) * 512],
                            start=(p == 0), stop=(p == 1),
                        )
                    if n == 0:
                        nc.vector.tensor_copy(ob[:, n * 512:(n + 1) * 512], pp[:])
                    else:
                        nc.scalar.activation(ob[:, n * 512:(n + 1) * 512], pp[:], COPY)
                # avoid the scalar ring: ACT's queue is backlogged with exps,
                # so its DMA dispatches would all bunch up at the very end
                eng = (nc.sync, nc.gpsimd)[t % 2]
                eng.dma_start(out[t * 128:(t + 1) * 128, :], ob[:])

            for t in range(4):
                emit_proj(t)
            for f in pending_norm2:  # norm2(1,3): after proj t0-3, before t12
                f()
            pending_norm2.clear()
            for t in range(4, KT):
                emit_proj(t)

    nc.compile()
    return nc


def _get_nc():
    global _NC_CACHE
    if _NC_CACHE is None:
        _NC_CACHE = _build_nc()
    return _NC_CACHE


def kernel(x, w_attn, b_attn, w_proj, b_proj, n_heads):
    import ml_dtypes
    bf16 = ml_dtypes.bfloat16

    x = np.asarray(x, dtype=np.float32)
    w_attn = np.asarray(w_attn, dtype=np.float32)
    b_attn = np.asarray(b_attn, dtype=np.float32)
    w_proj = np.asarray(w_proj, dtype=np.float32)
    b_proj = np.asarray(b_proj, dtype=np.float32)
    assert int(n_heads) == NH and x.shape == (B, T, C)

    # triangle: valid iff q - k = f - p >= 0 within the diagonal 128-block
    p_ = np.arange(128)[:, None]
    f_ = np.arange(128)[None, :]
    m1 = (f_ >= p_).astype(np.float32)
    maskd = np.ascontiguousarray(
        np.concatenate([m1, m1], axis=1).astype(bf16))
    ind2 = np.zeros((1, 256), dtype=np.float32)
    ind2[0, 0:64] = 1.0       # cols 0-127: indicator for h0 (rows 0-63)
    ind2[0, 192:256] = 1.0    # cols 128-255: indicator for h1 (rows 64-127)

    in_maps = []
    for core in range(NCORES):
        b, hg = core // 4, core % 4
        cs = hg * CS
        wq = w_attn[:, cs:cs + CS]
        wk = w_attn[:, C + cs:C + cs + CS]
        wv = w_attn[:, 2 * C + cs:2 * C + cs + CS]
        bq = b_attn[cs:cs + CS]
        bk = b_attn[C + cs:C + cs + CS]
        in_maps.append({
            "xT": np.ascontiguousarray(x[b].T.astype(bf16)),
            "wqkv": np.ascontiguousarray(
                np.concatenate([wq, wk, wv], axis=1).astype(bf16)
                .reshape(8, 128, 3 * CS).transpose(1, 0, 2)),
            "bqk": np.ascontiguousarray(
                np.stack([bq[:128], bq[128:], bk[:128], bk[128:]], axis=1)),
            "wproj": np.ascontiguousarray(w_proj[cs:cs + CS, :].astype(bf16)),
            "maskd": maskd,
            "ind2": ind2,
        })

    nc = _get_nc()
    trace = bool(os.environ.get("BASS_TRACE")) and _register_ntff_hook()
    res = run_bass_kernel_spmd(
        nc, in_maps, core_ids=list(range(NCORES)), trace=trace,
    )
    globals()["_LAST_RESULTS"] = res

    # host gather: sum head-group partials per batch, add adjusted bias
    # (v-bias folds through attention+proj into a constant row: b_v @ w_proj)
    b_eff = (b_proj.astype(np.float64)
             + b_attn[2 * C:].astype(np.float64) @ w_proj.astype(np.float64))
    outp = np.zeros((B, T, C), dtype=np.float64)
    for core in range(NCORES):
        outp[core // 4] += np.asarray(res.results[core]["out"]).astype(np.float64)
    outp += b_eff[None, None, :]
    return outp.astype(np.float32)


# revision 43
# speedup vs baseline: 1.0249x; 1.0067x over previous
"""Causal self-attention (B=2, T=2048, C=1024, NH=16) on 8 Trainium2 NeuronCores.

Sharding: core = (batch b, head-group hg): b = core//4, hg = core%4.
Each core handles batch b and 4 heads [4*hg, 4*hg+4) as two head-PAIRS,
computing a partial projection output (w_proj row-parallel). Host sums the
4 partials per batch and adds the (adjusted) bias.

v2 design (vs baseline): everything bf16 on-chip, S^T row-tiled so both
heads of a pair run CONCURRENTLY in the PE array (K=64 each, tile_position
(0,0)/(64,0)), causal-ragged S/exp/PV (only valid columns computed), exp of
both heads in one ACT instruction, denominator ones-column -> DVE reciprocal
-> K=2 indicator broadcast matmul -> in-place yT normalize. The projection
(qk/v) chains are software-pipelined INTO the ACT-paced attention phases as
PE filler so the HAM clock gate stays at K=8/8 (2.4 GHz).
"""

import os
import numpy as np
from contextlib import ExitStack

import concourse.bass as bass
import concourse.tile as tile
from concourse import bacc, mybir
from concourse.bass_utils import run_bass_kernel_spmd

F32 = mybir.dt.float32
F32R = mybir.dt.float32r
BF16 = mybir.dt.bfloat16
EXP = mybir.ActivationFunctionType.Exp
COPY = mybir.ActivationFunctionType.Copy

B, T, C = 2, 2048, 1024
NH, HD = 16, 64
NCORES = 8
HPC = 4            # heads per core
CS = HPC * HD      # 256 channels per core (per q/k/v)
KT = T // 128      # 16 k-tiles
NJ = T // 512      # 4 q-chunks
SCALE = 1.0 / np.sqrt(HD)

_NC_CACHE = None


def _register_ntff_hook():
    """The agent image's ``antenv`` lacks ``axon_hooks``; inject it and
    register the ctypes NTFF profiling hook so trace=True yields timings."""
    try:
        import sys, types, importlib
        if "antenv.axon_hooks" in sys.modules:
            return True
        tb = importlib.import_module("trn_agent_boot.trn_boot")
        hook = tb._ntff_profile_via_ctypes("/opt/axon/libaxon_pjrt.so")
        if hook is None:
            return False
        mod = types.ModuleType("antenv.axon_hooks")
        state = {"hook": hook}
        mod.set_axon_ntff_profile_hook = lambda h: state.update(hook=h)
        mod.get_axon_ntff_profile_hook = lambda: state["hook"]
        sys.modules["antenv.axon_hooks"] = mod
        import antenv
        antenv.axon_hooks = mod
        return True
    except Exception:
        return False


def _build_nc():
    nc = bacc.Bacc("TRN2", target_bir_lowering=False, debug=False)

    xT = nc.dram_tensor("xT", [C, T], BF16, kind="ExternalInput").ap()
    # host-packed [p, k, c] so the big weight DMA is fully contiguous
    wqkv = nc.dram_tensor("wqkv", [128, 8, 3 * CS], BF16, kind="ExternalInput").ap()
    bqk = nc.dram_tensor("bqk", [128, 4], F32, kind="ExternalInput").ap()
    wproj = nc.dram_tensor("wproj", [CS, C], BF16, kind="ExternalInput").ap()
    maskd = nc.dram_tensor("maskd", [128, 256], BF16, kind="ExternalInput").ap()
    ind2 = nc.dram_tensor("ind2", [1, 256], F32R, kind="ExternalInput").ap()
    out = nc.dram_tensor("out", [T, C], BF16, kind="ExternalOutput").ap()
    dbg = os.environ.get("BASS_DEBUG_DUMP")
    if dbg:
        dbg_d = nc.dram_tensor("dbg_d", [8, 1024], F32, kind="ExternalOutput").ap()
        dbg_dinv = nc.dram_tensor("dbg_dinv", [8, 1024], F32, kind="ExternalOutput").ap()
        dbg_yT = nc.dram_tensor("dbg_yT", [2, 128, T], F32, kind="ExternalOutput").ap()

    with tile.TileContext(nc) as tc:
        with ExitStack() as ctx:
            # ---- persistent sbuf ----
            pers = ctx.enter_context(tc.tile_pool(name="pers", bufs=1))
            xT_sb = [pers.tile([128, T], BF16, tag=f"xT{k}", name=f"xT{k}") for k in range(8)]
            w_big = pers.tile([128, 8, 3 * CS], BF16, tag="w_big")
            w_sb = [w_big[:, k, :] for k in range(8)]
            # qkT m-tiles: m0=q(pair0: h0|h1) m1=q(pair1) m2=k(pair0) m3=k(pair1)
            qkT = [pers.tile([128, T], BF16, tag=f"qkT{m}", name=f"qkT{m}") for m in range(4)]
            # v_aug: [128 k-rows, head, kt, 65]; col 64 = ones (denominator)
            v_sb = pers.tile([128, HPC, KT, 65], BF16, tag="v_sb")
            yT = [pers.tile([128, T], BF16, tag=f"yT{p}", name=f"yT{p}") for p in range(2)]
            wproj_sb = [pers.tile([128, C], BF16, tag=f"wproj{p}", name=f"wproj{p}") for p in range(2)]
            bqk_sb = pers.tile([128, 4], F32, tag="bqk_sb")
            maskd_sb = pers.tile([128, 256], BF16, tag="maskd_sb")
            # row 64 only: keeps the 1/d path lane-aligned with the po
            # denominator row (custom-DVE ops cannot cross partitions)
            ind2_sb = pers.tile([65, 256], F32R, tag="ind2_sb")

            nc.vector.memset(v_sb[:, :, :, 64], 1.0)
            # Two HWDGE rings in parallel (SP + ACT); each dma_start costs
            # ~0.6us of serialized dispatch on its ring, so coalesce.
            for k in range(8):   # per-k chunks: the k=0 slices land early so
                # the first qk chain (and attention) starts ASAP
                nc.sync.dma_start(w_big[:, k, :], wqkv[:, k, :])
                (nc.gpsimd, nc.scalar)[k % 2].dma_start(
                    xT_sb[k][:], xT[k * 128:(k + 1) * 128, :])
            for p in range(2):
                nc.sync.dma_start(wproj_sb[p][:], wproj[p * 128:(p + 1) * 128, :])
            nc.scalar.dma_start(bqk_sb[:], bqk[:])
            nc.scalar.dma_start(maskd_sb[:], maskd[:])
            nc.scalar.dma_start(ind2_sb[64:65, :], ind2[:])

            # ---- pools ----
            att = ctx.enter_context(tc.tile_pool(name="att", bufs=1))
            ctx2 = ctx.enter_context(ExitStack())
            psum = ctx2.enter_context(tc.tile_pool(name="psum", bufs=1, space="PSUM"))
            # psum budget (8 banks): spair 2x2 + po 2x1 + fill 2x1 = 8

            # ---- emission helpers ----
            def emit_qk_chain(m, j):
                pq = psum.tile([128, 512], F32, tag="fill", bufs=2, name=f"pq_{m}_{j}")
                for k in range(8):
                    nc.tensor.matmul(
                        pq[:],
                        w_sb[k][:, m * 128:(m + 1) * 128],
                        xT_sb[k][:, j * 512:(j + 1) * 512],
                        start=(k == 0), stop=(k == 7),
                    )
                nc.vector.tensor_scalar_add(
                    qkT[m][:, j * 512:(j + 1) * 512], pq[:], bqk_sb[:, m:m + 1]
                )

            def emit_v_chain(t):
                pv = psum.tile([128, 256], F32, tag="fill", bufs=2, name=f"pv_{t}")
                for k in range(8):
                    nc.tensor.matmul(
                        pv[:],
                        xT_sb[k][:, t * 128:(t + 1) * 128],
                        w_sb[k][:, 2 * CS:3 * CS],
                        start=(k == 0), stop=(k == 7),
                    )
                nc.vector.tensor_copy(
                    v_sb[:, :, t, 0:64],
                    pv[:].rearrange("p (h d) -> p h d", h=HPC),
                )

            dpool = ctx.enter_context(tc.tile_pool(name="dpool", bufs=1))

            def emit_norm(p, j, po_t):
                """PV for (p,j) done: reciprocal of the two denominator rows
                (fast Newton approx, ~18 bits), and evacuate unnormalized O^T
                into yT (bf16)."""
                dscr = dpool.tile([65, 1024], F32, tag="dscr", bufs=2, name=f"dscr_{p}_{j}")
                dinv = dpool.tile([65, 1024], F32R, tag="dinv", bufs=2, name=f"dinv_{p}_{j}")
                for h2 in range(2):
                    # tracked copy first: strict-FIFO DVE then guarantees the
                    # (custom-op) approx below sees the completed accumulation
                    nc.vector.tensor_copy(
                        yT[p][64 * h2:64 * h2 + 64, j * 512:(j + 1) * 512],
                        po_t[h2][0:64, :],
                    )
                    # full 65-partition approx: base partition 0 (custom-DVE
                    # ops are lane-fixed and only correct at base 0); rows
                    # 0-63 produce unused 1/O junk, row 64 = 1/d.
                    nc.vector.reciprocal_approx_fast(
                        dscr[:, 512 * h2:512 * h2 + 512], po_t[h2][:, :]
                    )
                with nc.allow_low_precision(reason="1/d fits tf32"):
                    nc.vector.tensor_copy(dinv[64:65, :], dscr[64:65, :])
                if dbg:
                    dr = dpool.tile([65, 1024], F32, tag="dbgd", bufs=2, name=f"dr_{p}_{j}")
                    for h2 in range(2):
                        nc.vector.tensor_copy(
                            dr[64:65, 512 * h2:512 * h2 + 512], po_t[h2][64:65, :])
                    nc.sync.dma_start(dbg_d[4 * p + j:4 * p + j + 1, :], dr[64:65, :])
                    nc.sync.dma_start(
                        dbg_dinv[4 * p + j:4 * p + j + 1, :], dscr[64:65, :])
                return dinv

            def emit_norm2(p, j, dinv):
                """Broadcast 1/d across the pair's 128 channel rows (K=2
                indicator matmul) and normalize yT in place."""
                db = psum.tile([128, 512], F32, tag="fill", bufs=2, name=f"db_{p}_{j}")
                for h2 in range(2):
                    nc.tensor.matmul(
                        db[:],
                        ind2_sb[64:65, 128 * h2:128 * h2 + 128],
                        dinv[64:65, 512 * h2:512 * h2 + 512],
                        start=(h2 == 0), stop=(h2 == 1),
                    )
                nc.vector.tensor_mul(
                    yT[p][:, j * 512:(j + 1) * 512],
                    yT[p][:, j * 512:(j + 1) * 512],
                    db[:],
                )

            # ---- attention (per pair), with PE filler interleave ----
            mask3 = maskd_sb[:].rearrange("p (c b) -> p c b", c=2)

            def attention_pair(p, fillers, pending_norm2):
                """fillers: list of (need_step, thunk), sorted by need_step.
                Popped when due (data needed soon) or on a 1-in-3 step pace
                to keep the PE stream dense through the ACT-paced phase."""
                step = 0
                for j in range(NJ):
                    last = 4 * j + 3
                    po_t = None
                    pend = []   # PV lags S/exp by 2 steps: never waits on ACT
                    for kt in range(last + 1):
                        while fillers and fillers[0][0] <= step:
                            fillers.pop(0)[1]()
                        d = max(0, kt - 4 * j)
                        w = 512 - 128 * d
                        qoff = j * 512 + 128 * d
                        spair = psum.tile([128, 1024], F32, tag="spair", bufs=2,
                                          name=f"sp_{p}_{j}_{kt}")
                        sp3 = spair.rearrange("p (c b) -> p c b", c=2)
                        for h2 in range(2):
                            nc.tensor.matmul(
                                sp3[:, h2, 0:w],
                                qkT[2 + p][64 * h2:64 * h2 + 64, kt * 128:(kt + 1) * 128],
                                qkT[p][64 * h2:64 * h2 + 64, qoff:qoff + w],
                                start=True, stop=True,
                                tile_position=(64 * h2, 0),
                            )
                        # flush the oldest pending PV (2-step lag)
                        if len(pend) >= 2:
                            kt0, pt0, w0, d0 = pend.pop(0)
                            for h2 in range(2):
                                nc.tensor.matmul(
                                    po_t[h2][:, 128 * d0:512],
                                    v_sb[:, 2 * p + h2, kt0, :],
                                    pt0[:, h2, 0:w0],
                                    start=(kt0 == 0), stop=(kt0 == last),
                                )
                        if step % 3 == 1 and fillers:
                            fillers.pop(0)[1]()
                        if kt == 2 and pending_norm2:
                            pending_norm2.pop(0)()
                        pt = att.tile([128, 1024], BF16, tag="pt", bufs=3,
                                      name=f"pt_{p}_{j}_{kt}")
                        pt3 = pt.rearrange("p (c b) -> p c b", c=2)
                        nc.scalar.activation(pt3[:, :, 0:w], sp3[:, :, 0:w], EXP, scale=SCALE)
                        if kt >= 4 * j:  # diagonal block: triangular mask
                            nc.vector.tensor_mul(
                                pt3[:, :, 0:128], pt3[:, :, 0:128], mask3
                            )
                        if po_t is None:
                            po_t = [psum.tile([65, 512], F32, tag="po", bufs=2,
                                              name=f"po_{p}_{j}_{h2}")
                                    for h2 in range(2)]
                        pend.append((kt, pt3, w, d))
                        step += 1
                    # flush remaining PVs of this j
                    for kt0, pt0, w0, d0 in pend:
                        for h2 in range(2):
                            nc.tensor.matmul(
                                po_t[h2][:, 128 * d0:512],
                                v_sb[:, 2 * p + h2, kt0, :],
                                pt0[:, h2, 0:w0],
                                start=(kt0 == 0), stop=(kt0 == last),
                            )
                    pend = []
                    dinv = emit_norm(p, j, po_t)
                    pending_norm2.append(lambda p=p, j=j, dinv=dinv: emit_norm2(p, j, dinv))

            # ---- phase 1 (minimal): only what A0's first steps consume ----
            emit_qk_chain(0, 0)
            emit_qk_chain(2, 0)
            for t in range(4):
                emit_v_chain(t)

            pending_norm2 = []
            # step(j, kt) = base(j) + kt;  base = [0, 4, 12, 24]
            base = [0, 4, 12, 24]
            # ---- A0: pair0 attention; fillers = v[4..15] ----
            # v(t) first consumed by PV(kt=t) at step base(j0)+t+1
            fillers0 = []
            for t in range(4, KT):
                j0 = t // 4  # first j whose kt range reaches t
                fillers0.append((base[j0] + t - 1, lambda t=t: emit_v_chain(t)))
            # pair0's remaining qk chains, just-in-time (qkT[0] chunk c read
            # by S(c,0); qkT[2] chunk c read by S(c,4c))
            for c in range(1, NJ):
                fillers0.append((base[c] - 2, lambda c=c: emit_qk_chain(0, c)))
                fillers0.append((base[c] + 4 * c - 2, lambda c=c: emit_qk_chain(2, c)))
            # pair1's first qk chunks late in A0 so A1 starts without a stall
            fillers0.append((28, lambda: emit_qk_chain(1, 0)))
            fillers0.append((31, lambda: emit_qk_chain(3, 0)))
            fillers0.sort(key=lambda x: x[0])
            attention_pair(0, fillers0, pending_norm2)
            for _, f in fillers0:
                f()
            # ---- A1: pair1 attention; fillers = remaining qk chains ----
            # qk(1,c) read by S(c, 0); qk(3,c) read by S(c, 4c)
            fillers1 = []
            for c in range(1, NJ):
                fillers1.append((base[c] - 2, lambda c=c: emit_qk_chain(1, c)))
                fillers1.append((base[c] + 4 * c - 2, lambda c=c: emit_qk_chain(3, c)))
            fillers1.sort(key=lambda x: x[0])
            attention_pair(1, fillers1, pending_norm2)
            for _, f in fillers1:
                f()

            if dbg:
                for p in range(2):
                    yf = att.tile([128, T], F32, tag="dbgy", bufs=1, name=f"yf_{p}")
                    nc.vector.tensor_copy(yf[:], yT[p][:])
                    nc.sync.dma_start(dbg_yT[p, :, :], yf[:])

            # ---- phase 3: projection (contraction over both pairs) ----
            # pp reuses the attention "spair" psum tag (pools stay open so the
            # deferred norm2(1,3) db matmul can still allocate from "fill")
            def emit_proj(t):
                ob = att.tile([128, C], BF16, tag="ob", bufs=4, name=f"ob_{t}")
                for n in range(2):
                    pp = psum.tile([128, 512], F32, tag="spair", bufs=2, name=f"pp_{t}_{n}")
                    for p in range(2):
                        nc.tensor.matmul(
                            pp[:],
                            yT[p][:, t * 128:(t + 1) * 128],
                            wproj_sb[p][:, n * 512:(n + 1) * 512],
                            start=(p == 0), stop=(p == 1),
                        )
                    if n == 0:
                        nc.vector.tensor_copy(ob[:, n * 512:(n + 1) * 512], pp[:])
                    else:
                        nc.scalar.activation(ob[:, n * 512:(n + 1) * 512], pp[:], COPY)
                # avoid the scalar ring: ACT's queue is backlogged with exps,
                # so its DMA dispatches would all bunch up at the very end
                eng = (nc.sync, nc.gpsimd)[t % 2]
                eng.dma_start(out[t * 128:(t + 1) * 128, :], ob[:])

            for t in range(4):
                emit_proj(t)
            for f in pending_norm2:  # norm2(1,3): after proj t0-3, before t12
                f()
            pending_norm2.clear()
            for t in range(4, KT):
                emit_proj(t)

    nc.compile()
    return nc


def _get_nc():
    global _NC_CACHE
    if _NC_CACHE is None:
        _NC_CACHE = _build_nc()
    return _NC_CACHE


def kernel(x, w_attn, b_attn, w_proj, b_proj, n_heads):
    import ml_dtypes
    bf16 = ml_dtypes.bfloat16

    x = np.asarray(x, dtype=np.float32)
    w_attn = np.asarray(w_attn, dtype=np.float32)
    b_attn = np.asarray(b_attn, dtype=np.float32)
    w_proj = np.asarray(w_proj, dtype=np.float32)
    b_proj = np.asarray(b_proj, dtype=np.float32)
    assert int(n_heads) == NH and x.shape == (B, T, C)

    # triangle: valid iff q - k = f - p >= 0 within the diagonal 128-block
    p_ = np.arange(128)[:, None]
    f_ = np.arange(128)[None, :]
    m1 = (f_ >= p_).astype(np.float32)
    maskd = np.ascontiguousarray(
        np.concatenate([m1, m1], axis=1).astype(bf16))
    ind2 = np.zeros((1, 256), dtype=np.float32)
    ind2[0, 0:64] = 1.0       # cols 0-127: indicator for h0 (rows 0-63)
    ind2[0, 192:256] = 1.0    # cols 128-255: indicator for h1 (rows 64-127)

    in_maps = []
    for core in range(NCORES):
        b, hg = core // 4, core % 4
        cs = hg * CS
        wq = w_attn[:, cs:cs + CS]
        wk = w_attn[:, C + cs:C + cs + CS]
        wv = w_attn[:, 2 * C + cs:2 * C + cs + CS]
        bq = b_attn[cs:cs + CS]
        bk = b_attn[C + cs:C + cs + CS]
        in_maps.append({
            "xT": np.ascontiguousarray(x[b].T.astype(bf16)),
            "wqkv": np.ascontiguousarray(
                np.concatenate([wq, wk, wv], axis=1).astype(bf16)
                .reshape(8, 128, 3 * CS).transpose(1, 0, 2)),
            "bqk": np.ascontiguousarray(
                np.stack([bq[:128], bq[128:], bk[:128], bk[128:]], axis=1)),
            "wproj": np.ascontiguousarray(w_proj[cs:cs + CS, :].astype(bf16)),
            "maskd": maskd,
            "ind2": ind2,
        })

    nc = _get_nc()
    trace = bool(os.environ.get("BASS_TRACE")) and _register_ntff_hook()
    res = run_bass_kernel_spmd(
        nc, in_maps, core_ids=list(range(NCORES)), trace=trace,
    )
    globals()["_LAST_RESULTS"] = res

    # host gather: sum head-group partials per batch, add adjusted bias
    # (v-bias folds through attention+proj into a constant row: b_v @ w_proj)
    b_eff = (b_proj.astype(np.float64)
             + b_attn[2 * C:].astype(np.float64) @ w_proj.astype(np.float64))
    outp = np.zeros((B, T, C), dtype=np.float64)
    for core in range(NCORES):
        outp[core // 4] += np.asarray(res.results[core]["out"]).astype(np.float64)
    outp += b_eff[None, None, :]
    return outp.astype(np.float32)
